# revision 23
# baseline (speedup 1.0000x reference)
"""Trainium2 Bass kernel for MissingSensorImputation (GNN message passing).

Strategy (per core, data-parallel over batch: 8 cores x 2 batches):
  - One SWDGE casting DMA reorganizes x into a bf16 HBM table [node, 2*H]
    (512B rows) for gathering.
  - Edges sorted by dst, grouped per 128-node dst block, padded to chunks of
    128 edges. dma_gather pulls each chunk's src rows into SBUF partitions.
  - Scatter-add = PE matmul: onehot(dstlocal)^T @ msgs accumulated in PSUM
    per dst block (onehot generated on DVE via iota==dstlocal compare in
    2x_1p-friendly paired layout).
  - MLP (feature-major): transpose neighbor and x per block on PE,
    h=relu(W1a@nT + W1b@xT + b1), imputed = W2@h + b2, transpose back,
    blend with resident fp32 x by the missing mask.
Host-side preprocessing touches only the edge index (sorting/padding) and
constant tables; all tensor math runs on device.
"""
import sys

sys.path.insert(0, "/opt/trn_rl_repo")

import numpy as np
import concourse.bacc as bacc
import concourse.mybir as mybir
import concourse.tile as tile
from concourse import bass_utils

P = 128
NCORES = 8
F32 = mybir.dt.float32
BF16 = mybir.dt.bfloat16
I16 = mybir.dt.int16
I32 = mybir.dt.int32
NPBF16 = mybir.dt.np(BF16)

AF = mybir.ActivationFunctionType
ALU = mybir.AluOpType

_cache = {}
last_results = None  # BassKernelResults of the most recent run (for profiling)


def _prep_edges(edge_index, N, keeps):
    """Sort edges by dst, group per 128-node dst block, pad to 128-multiples.

    keeps: per-core boolean [N] — edges whose dst is unmasked in every batch
    of that core are dead (the reference discards imputed there) and are
    dropped. Chunk counts are padded to a common per-block max so all cores
    run one program; padding edges read the zero row (index N), dstlocal 0.

    Returns (nch, idx_list, dl_list): common chunks per block, and per-core
    wrapped int16 gather indices [128, Ctot*8] plus dstlocal pair columns
    [128, Ctot, 2] bf16 (values duplicated for the DVE 2x perf mode).
    """
    NB = (N + P - 1) // P
    src = np.asarray(edge_index[0], np.int64)
    dst = np.asarray(edge_index[1], np.int64)
    order = np.argsort(dst, kind="stable")
    ss, ds = src[order], dst[order]
    per_core = []
    for keep in keeps:
        sel = keep[ds]
        ss_c, ds_c = ss[sel], ds[sel]
        bounds = np.searchsorted(ds_c, np.arange(0, (NB + 1) * P, P).clip(max=N))
        per_core.append((ss_c, ds_c, bounds))
    nch = []
    for b in range(NB):
        mx = max(pc[2][b + 1] - pc[2][b] for pc in per_core)
        nch.append(max(1, -(-int(mx) // P)))
    ctot = sum(nch)
    idx_list, dl_list = [], []
    for ss_c, ds_c, bounds in per_core:
        srcs, dls = [], []
        for b in range(NB):
            lo, hi = bounds[b], bounds[b + 1]
            s_b = ss_c[lo:hi]
            d_b = ds_c[lo:hi] - b * P
            npad = nch[b] * P - len(s_b)
            srcs.append(np.concatenate([s_b, np.full(npad, N, np.int64)]))
            dls.append(np.concatenate([d_b, np.zeros(npad, np.int64)]))
        src_all = np.concatenate(srcs).astype(np.int16)
        dl_all = np.concatenate(dls).astype(np.float32)
        iw = src_all.reshape(ctot * 8, 16).T  # index i -> [i%16, i//16]
        idx_list.append(np.tile(iw, (8, 1)).copy())
        dl_cols = dl_all.reshape(ctot, P).T  # [p, c] = dl[c*128+p]
        dl_list.append(
            np.repeat(dl_cols[:, :, None], 2, axis=2).astype(NPBF16).copy()
        )
    return tuple(nch), idx_list, dl_list


def _build(nch, B, N, H):
    """Build the Bass program (same for all cores)."""
    BPC = B // NCORES
    NB = (N + P - 1) // P
    NPAD = N + 16
    FD = BPC * H  # free-dim row size per node (256)
    ctot = sum(nch)
    coloff = np.concatenate([[0], np.cumsum(nch)]).astype(int)
    NBF = N // P  # full blocks
    rem = N - NBF * P

    nc = bacc.Bacc("TRN2", target_bir_lowering=False, debug=False, num_swdge_queues=2)

    x_in = nc.dram_tensor("x", [BPC, N, H], F32, kind="ExternalInput").ap()
    m_in = nc.dram_tensor("mask", [BPC, N], I32, kind="ExternalInput").ap()
    w1_in = nc.dram_tensor("W1", [2 * H, H], F32, kind="ExternalInput").ap()
    b1_in = nc.dram_tensor("b1", [H], F32, kind="ExternalInput").ap()
    w2_in = nc.dram_tensor("W2", [H, H], F32, kind="ExternalInput").ap()
    b2_in = nc.dram_tensor("b2", [H], F32, kind="ExternalInput").ap()
    idx_in = nc.dram_tensor("idx", [P, ctot * 8], I16, kind="ExternalInput").ap()
    dl_in = nc.dram_tensor("dl", [P, ctot, 2], BF16, kind="ExternalInput").ap()
    iota_in = nc.dram_tensor("iota", [P, P], BF16, kind="ExternalInput").ap()
    idb_in = nc.dram_tensor("idb", [P, P], BF16, kind="ExternalInput").ap()
    idf_in = nc.dram_tensor("idf", [P, P], F32, kind="ExternalInput").ap()
    y_out = nc.dram_tensor("y", [BPC, N, H], F32, kind="ExternalOutput").ap()

    with tile.TileContext(nc) as tc:
        with (
            tc.tile_pool(name="const", bufs=1) as cpool,
            tc.tile_pool(name="work", bufs=3) as wpool,
            tc.tile_pool(name="msgs", bufs=2) as mpool,
            tc.tile_pool(name="ohp", bufs=2) as opool,
            tc.tile_pool(name="idxp", bufs=3) as ipool,
            tc.tile_pool(name="psA", bufs=2, space="PSUM") as psA,
            tc.tile_pool(name="psB", bufs=1, space="PSUM") as psB,
            tc.tile_pool(name="dram", bufs=1, space="DRAM") as dpool,
        ):
            # ---- constants / persistent tiles ----
            dl_t = cpool.tile([P, ctot, 2], BF16, tag="dl")
            nc.sync.dma_start(out=dl_t[:], in_=dl_in)
            iota_t = cpool.tile([P, P], BF16, tag="iota")
            nc.sync.dma_start(out=iota_t[:], in_=iota_in)
            idb_t = cpool.tile([P, P], BF16, tag="idb")
            nc.sync.dma_start(out=idb_t[:], in_=idb_in)
            idf_t = cpool.tile([P, P], F32, tag="idf")
            nc.sync.dma_start(out=idf_t[:], in_=idf_in)

            w1a_f = cpool.tile([P, H], F32, tag="w1af")
            nc.sync.dma_start(out=w1a_f[:], in_=w1_in[0:H, :])
            w1b_f = cpool.tile([P, H], F32, tag="w1bf")
            nc.sync.dma_start(out=w1b_f[:], in_=w1_in[H:2 * H, :])
            w2_f = cpool.tile([P, H], F32, tag="w2f")
            nc.sync.dma_start(out=w2_f[:], in_=w2_in)

            b1c = cpool.tile([P, 1], F32, tag="b1c")
            nc.sync.dma_start(out=b1c[:], in_=b1_in[:, None])
            b2c = cpool.tile([P, 1], F32, tag="b2c")
            nc.sync.dma_start(out=b2c[:], in_=b2_in[:, None])

            # mask -> per-block per-partition columns, f32, and (1-m)
            mi = cpool.tile([P, NB, BPC], I32, tag="mi")
            for j in range(BPC):
                nc.sync.dma_start(
                    out=mi[:, 0:NBF, j],
                    in_=m_in[j, 0:NBF * P].rearrange("(nb p) -> p nb", p=P),
                )
                if rem:
                    nc.sync.dma_start(
                        out=mi[:rem, NBF, j], in_=m_in[j, NBF * P:N][:, None]
                    )
            m_f = cpool.tile([P, NB, BPC], F32, tag="mf")
            nc.vector.tensor_copy(out=m_f[:], in_=mi[:])
            omm = cpool.tile([P, NB, BPC], F32, tag="omm")
            nc.vector.tensor_scalar(
                out=omm[:], in0=m_f[:], scalar1=-1.0, scalar2=1.0,
                op0=ALU.mult, op1=ALU.add,
            )

            # resident fp32 x, node-major per block: [p, nb, batch, h].
            # Loaded in row ranges; each range is cast to the bf16 gather
            # table as soon as its batches land, overlapping load and cast.
            x_res = cpool.tile([P, NB, BPC, H], F32, tag="xres")
            xbf = dpool.tile([NPAD, BPC, H], BF16, tag="xbf")
            RSPLIT = 4
            edges_r = [NBF * (i + 1) // RSPLIT for i in range(RSPLIT)]
            lo = 0
            for hi in edges_r:
                for j in range(BPC):
                    nc.sync.dma_start(
                        out=x_res[:, lo:hi, j, :],
                        in_=x_in[j, lo * P:hi * P, :].rearrange(
                            "(nb p) h -> p nb h", p=P
                        ),
                    )
                nc.gpsimd.dma_start(
                    out=xbf[lo * P:hi * P, :, :].rearrange(
                        "(nb p) b h -> p nb (b h)", p=P
                    ),
                    in_=x_res[:, lo:hi, :, :].rearrange("p nb b h -> p nb (b h)"),
                )
                lo = hi
            if rem:
                for j in range(BPC):
                    nc.sync.dma_start(
                        out=x_res[:rem, NBF, j, :],
                        in_=x_in[j, NBF * P:N, :],
                    )
                nc.gpsimd.dma_start(
                    out=xbf[NBF * P:N, :, :].rearrange("p b h -> p (b h)"),
                    in_=x_res[:rem, NBF, :, :].rearrange("p b h -> p (b h)"),
                )
            zrow = cpool.tile([16, FD], BF16, tag="zrow")
            nc.gpsimd.memset(zrow[:], 0.0)
            nc.sync.dma_start(
                out=xbf[N:NPAD, :, :].rearrange("r b h -> r (b h)"), in_=zrow[:]
            )

            # ---- main loop over dst blocks (gathers merged in pairs) ----
            xbf_flat = xbf[:].rearrange("r b h -> r (b h)")
            IDXG = 4  # blocks per idx-load DMA
            idx_tiles = {}
            pairs = [
                tuple(pb_ for pb_ in (b0, b0 + 1) if pb_ < NB)
                for b0 in range(0, NB, 2)
            ]
            for pair in pairs:
                b0 = pair[0]
                kp = sum(nch[b] for b in pair)
                poff = int(coloff[b0])

                if b0 % IDXG == 0:
                    ge = min(NB, b0 + IDXG)
                    goff = int(coloff[b0])
                    gk = int(coloff[ge]) - goff
                    gt = ipool.tile([P, gk * 8], I16, tag="idxs")
                    nc.sync.dma_start(
                        out=gt[:], in_=idx_in[:, goff * 8:(goff + gk) * 8]
                    )
                    idx_tiles = {"tile": gt, "goff": goff}
                loc = (poff - idx_tiles["goff"]) * 8
                ms = mpool.tile([P, kp, FD], BF16, tag="ms")
                nc.gpsimd.dma_gather(
                    out_ap=ms[:],
                    in_ap=xbf_flat,
                    idxs_ap=idx_tiles["tile"][:, loc:loc + kp * 8],
                    num_idxs=kp * P,
                    num_idxs_reg=kp * P,
                    elem_size=FD,
                    single_packet=False,
                    queue_num=(b0 // 2) % 2,
                )
                oh = opool.tile([P, kp, P], BF16, tag="oh")
                nc.vector.tensor_tensor(
                    out=oh[:].rearrange("p c (a t) -> p c a t", t=2),
                    in0=iota_t[:, None, :].to_broadcast([P, kp, P]).rearrange(
                        "p c (a t) -> p c a t", t=2
                    ),
                    in1=dl_t[:, poff:poff + kp, None, :].to_broadcast(
                        [P, kp, P // 2, 2]
                    ),
                    op=ALU.is_equal,
                )

                for b in pair:
                    c0 = b * P
                    pb = min(P, N - c0)
                    k = nch[b]
                    lo = int(coloff[b]) - poff

                    nbp = psA.tile([P, BPC, H], F32, tag="nbp", space="PSUM")
                    for c in range(k):
                        nc.tensor.matmul(
                            out=nbp[:].rearrange("p b h -> p (b h)"),
                            lhsT=oh[:, lo + c, :],
                            rhs=ms[:, lo + c, :],
                            start=(c == 0),
                            stop=(c == k - 1),
                        )
                    nb_sb = wpool.tile([P, BPC, H], F32, tag="nb_sb")
                    nc.scalar.activation(out=nb_sb[:], in_=nbp[:], func=AF.Copy)

                    tpn = psB.tile([P, BPC, P], F32, tag="tpn", space="PSUM")
                    for j in range(BPC):
                        nc.tensor.transpose(
                            out=tpn[:, j, :], in_=nb_sb[:, j, :], identity=idf_t[:]
                        )
                    nT = wpool.tile([P, BPC, P], F32, tag="nT")
                    nc.scalar.activation(out=nT[:], in_=tpn[:], func=AF.Copy)

                    # per-block feature-major x from resident fp32 x
                    tpx = psB.tile([P, BPC, P], F32, tag="tpx", space="PSUM")
                    for j in range(BPC):
                        nc.tensor.transpose(
                            out=tpx[:, j, :pb], in_=x_res[:pb, b, j, :],
                            identity=idf_t[:pb, :pb],
                        )
                    xTb = wpool.tile([P, BPC, P], F32, tag="xTb")
                    nc.scalar.activation(
                        out=xTb[:, :, :pb], in_=tpx[:, :, :pb], func=AF.Copy
                    )

                    h_ps = psB.tile([P, BPC, H], F32, tag="h_ps", space="PSUM")
                    nc.tensor.matmul(
                        out=h_ps[:, :, :pb], lhsT=w1a_f[:], rhs=nT[:, :, :pb],
                        start=True, stop=False,
                    )
                    nc.tensor.matmul(
                        out=h_ps[:, :, :pb], lhsT=w1b_f[:], rhs=xTb[:, :, :pb],
                        start=False, stop=True,
                    )
                    h_sb = wpool.tile([P, BPC, H], F32, tag="h_sb")
                    nc.scalar.activation(
                        out=h_sb[:, :, :pb], in_=h_ps[:, :, :pb], func=AF.Relu,
                        bias=b1c[:],
                    )

                    imp_ps = psB.tile([P, BPC, H], F32, tag="imp_ps", space="PSUM")
                    nc.tensor.matmul(
                        out=imp_ps[:, :, :pb], lhsT=w2_f[:],
                        rhs=h_sb[:, :, :pb],
                        start=True, stop=True,
                    )
                    imp_sb = wpool.tile([P, BPC, H], F32, tag="imp_sb")
                    nc.vector.tensor_scalar(
                        out=imp_sb[:, :, :pb], in0=imp_ps[:, :, :pb],
                        scalar1=b2c[:], scalar2=None, op0=ALU.add,
                    )

                    outT = psA.tile([P, BPC, P], F32, tag="outT", space="PSUM")
                    for j in range(BPC):
                        nc.tensor.transpose(
                            out=outT[:pb, j, :], in_=imp_sb[:, j, :pb],
                            identity=idf_t[:],
                        )

                    xt_sb = wpool.tile([P, BPC, H], F32, tag="xt_sb")
                    for j in range(BPC):
                        nc.scalar.activation(
                            out=xt_sb[:pb, j, :], in_=x_res[:pb, b, j, :],
                            func=AF.Copy, scale=omm[:pb, b:b + 1, j],
                        )
                    out_sb = wpool.tile([P, BPC, H], F32, tag="out_sb")
                    for j in range(BPC):
                        nc.vector.scalar_tensor_tensor(
                            out=out_sb[:pb, j, :],
                            in0=outT[:pb, j, :],
                            scalar=m_f[:pb, b:b + 1, j],
                            in1=xt_sb[:pb, j, :],
                            op0=ALU.mult,
                            op1=ALU.add,
                        )
                    nc.sync.dma_start(
                        out=y_out[:, c0:c0 + pb, :].rearrange("b p h -> p b h"),
                        in_=out_sb[:pb],
                    )

    nc.compile()
    return nc


def kernel(node_embeddings, missing_mask, edge_index, W1, b1, W2, b2, trace=False):
    global last_results
    x = np.ascontiguousarray(np.asarray(node_embeddings, np.float32))
    mask = np.ascontiguousarray(np.asarray(missing_mask, np.int32))
    B, N, H = x.shape
    assert H == P and B % NCORES == 0
    BPC = B // NCORES

    ekey = (N, B, H, hash(np.asarray(edge_index).tobytes()),
            hash(mask.tobytes()))
    if ekey not in _cache:
        keeps = [
            (mask[c * BPC:(c + 1) * BPC] != 0).any(axis=0) for c in range(NCORES)
        ]
        nch, idx_list, dl_list = _prep_edges(edge_index, N, keeps)
        nc = _build(nch, B, N, H)
        _cache[ekey] = (nc, idx_list, dl_list)
    nc, idx_list, dl_list = _cache[ekey]

    iota_arr = np.broadcast_to(
        np.arange(P, dtype=np.float32), (P, P)
    ).astype(NPBF16).copy()
    idb_arr = np.eye(P, dtype=np.float32).astype(NPBF16)
    idf_arr = np.eye(P, dtype=np.float32)

    common = {
        "W1": np.ascontiguousarray(np.asarray(W1, np.float32)),
        "b1": np.ascontiguousarray(np.asarray(b1, np.float32)),
        "W2": np.ascontiguousarray(np.asarray(W2, np.float32)),
        "b2": np.ascontiguousarray(np.asarray(b2, np.float32)),
        "iota": iota_arr,
        "idb": idb_arr,
        "idf": idf_arr,
    }
    in_maps = []
    for c in range(NCORES):
        m = dict(common)
        m["x"] = np.ascontiguousarray(x[c * BPC:(c + 1) * BPC])
        m["mask"] = np.ascontiguousarray(mask[c * BPC:(c + 1) * BPC])
        m["idx"] = idx_list[c]
        m["dl"] = dl_list[c]
        in_maps.append(m)

    try:
        res = bass_utils.run_bass_kernel_spmd(
            nc, in_maps, core_ids=list(range(NCORES)), trace=trace
        )
    except ModuleNotFoundError:
        res = bass_utils.run_bass_kernel_spmd(
            nc, in_maps, core_ids=list(range(NCORES)), trace=False
        )
    last_results = res
    return np.concatenate([res.results[c]["y"] for c in range(NCORES)], axis=0)


# revision 24
# speedup vs baseline: 1.1086x; 1.1086x over previous
"""Trainium2 Bass kernel for MissingSensorImputation (GNN message passing).

Strategy (per core, data-parallel over batch: 8 cores x 2 batches):
  - One SWDGE casting DMA reorganizes x into a bf16 HBM table [node, 2*H]
    (512B rows) for gathering.
  - Edges sorted by dst, grouped per 128-node dst block, padded to chunks of
    128 edges. dma_gather pulls each chunk's src rows into SBUF partitions.
  - Scatter-add = PE matmul: onehot(dstlocal)^T @ msgs accumulated in PSUM
    per dst block (onehot generated on DVE via iota==dstlocal compare in
    2x_1p-friendly paired layout).
  - MLP (feature-major): transpose neighbor and x per block on PE,
    h=relu(W1a@nT + W1b@xT + b1), imputed = W2@h + b2, transpose back,
    blend with resident fp32 x by the missing mask.
Host-side preprocessing touches only the edge index (sorting/padding) and
constant tables; all tensor math runs on device.
"""
import sys

sys.path.insert(0, "/opt/trn_rl_repo")

import numpy as np
import concourse.bacc as bacc
import concourse.mybir as mybir
import concourse.tile as tile
from concourse import bass_utils

P = 128
NCORES = 8
F32 = mybir.dt.float32
BF16 = mybir.dt.bfloat16
I16 = mybir.dt.int16
I32 = mybir.dt.int32
NPBF16 = mybir.dt.np(BF16)

AF = mybir.ActivationFunctionType
ALU = mybir.AluOpType

_cache = {}
last_results = None  # BassKernelResults of the most recent run (for profiling)


def _slots(s_b, d_b, N):
    """Group a block's edges by src into slots carrying up to 3 dst columns.

    Returns (src, d1, d2, d3, n2, n3): slot source rows, up to three
    dstlocal values per slot (255 = unused sentinel), and the counts of
    slots having a 2nd / 3rd dst. Slots with more dsts sort first so d2/d3
    occupy a prefix.
    """
    o = np.argsort(s_b, kind="stable")
    sv, dv = s_b[o], d_b[o]
    n = len(sv)
    if n == 0:
        z = np.zeros(0, np.int64)
        return z, z, z, z, 0, 0
    new = np.r_[True, sv[1:] != sv[:-1]]
    runstart = np.maximum.accumulate(np.where(new, np.arange(n), 0))
    pos = np.arange(n) - runstart
    sub = pos // 3
    rank = pos % 3
    runid = np.cumsum(new) - 1
    skey = runid * (n + 1) + sub
    uniq, inv = np.unique(skey, return_inverse=True)
    ns = len(uniq)
    slot_src = np.zeros(ns, np.int64)
    d1 = np.zeros(ns, np.int64)
    d2 = np.full(ns, 255, np.int64)
    d3 = np.full(ns, 255, np.int64)
    slot_src[inv] = sv
    d1[inv[rank == 0]] = dv[rank == 0]
    d2[inv[rank == 1]] = dv[rank == 1]
    d3[inv[rank == 2]] = dv[rank == 2]
    okey = -((d3 != 255).astype(np.int64) * 2 + (d2 != 255).astype(np.int64))
    so = np.argsort(okey, kind="stable")
    return (slot_src[so], d1[so], d2[so], d3[so],
            int((d2 != 255).sum()), int((d3 != 255).sum()))


def _pack_pairs(vals, ctot):
    cols = vals.reshape(ctot, P).T  # [p, c]
    return np.repeat(cols[:, :, None], 2, axis=2).astype(NPBF16).copy()


def _prep_edges(edge_index, N, keeps):
    """Sort kept edges by dst block, dedup (block, src) into <=3-dst slots.

    keeps: per-core boolean [N] — edges whose dst is unmasked in every batch
    of that core are dead and dropped. Slot counts are padded to common
    per-block maxima so all cores run one program; padding slots read the
    zero row (index N) with d1=0 (zero contribution), d2=d3=255 (sentinel).

    Returns (nch, m2ch, m3ch, idx_list, dl1_list, dl2_list, dl3_list).
    """
    NB = (N + P - 1) // P
    src = np.asarray(edge_index[0], np.int64)
    dst = np.asarray(edge_index[1], np.int64)
    order = np.argsort(dst, kind="stable")
    ss, ds = src[order], dst[order]
    per_core = []
    for keep in keeps:
        sel = keep[ds]
        ss_c, ds_c = ss[sel], ds[sel]
        bounds = np.searchsorted(ds_c, np.arange(0, (NB + 1) * P, P).clip(max=N))
        blocks = []
        for b in range(NB):
            lo, hi = bounds[b], bounds[b + 1]
            blocks.append(_slots(ss_c[lo:hi], ds_c[lo:hi] - b * P, N))
        per_core.append(blocks)
    nch, m2ch, m3ch = [], [], []
    for b in range(NB):
        nch.append(max(1, max(-(-len(pc[b][0]) // P) for pc in per_core)))
        m2ch.append(max(-(-pc[b][4] // P) for pc in per_core))
        m3ch.append(max(-(-pc[b][5] // P) for pc in per_core))
    ctot, ctot2, ctot3 = sum(nch), sum(m2ch), sum(m3ch)
    idx_list, dl1_list, dl2_list, dl3_list = [], [], [], []
    for blocks in per_core:
        srcs, dl1s, dl2s, dl3s = [], [], [], []
        for b in range(NB):
            sv, d1, d2, d3, _, _ = blocks[b]
            npad = nch[b] * P - len(sv)
            srcs.append(np.concatenate([sv, np.full(npad, N, np.int64)]))
            d1f = np.concatenate([d1, np.zeros(npad, np.int64)])
            d2f = np.concatenate([d2, np.full(npad, 255, np.int64)])
            d3f = np.concatenate([d3, np.full(npad, 255, np.int64)])
            dl1s.append(d1f)
            dl2s.append(d2f[:m2ch[b] * P])
            dl3s.append(d3f[:m3ch[b] * P])
        src_all = np.concatenate(srcs).astype(np.int16)
        iw = src_all.reshape(ctot * 8, 16).T  # index i -> [i%16, i//16]
        idx_list.append(np.tile(iw, (8, 1)).copy())
        dl1_list.append(_pack_pairs(
            np.concatenate(dl1s).astype(np.float32), ctot))
        dl2_list.append(_pack_pairs(
            np.concatenate(dl2s).astype(np.float32), max(ctot2, 1))
            if ctot2 else np.zeros((P, 1, 2), NPBF16))
        dl3_list.append(_pack_pairs(
            np.concatenate(dl3s).astype(np.float32), max(ctot3, 1))
            if ctot3 else np.zeros((P, 1, 2), NPBF16))
    return (tuple(nch), tuple(m2ch), tuple(m3ch),
            idx_list, dl1_list, dl2_list, dl3_list)


def _build(nch, m2ch, m3ch, B, N, H):
    """Build the Bass program (same for all cores)."""
    BPC = B // NCORES
    NB = (N + P - 1) // P
    NPAD = N + 16
    FD = BPC * H  # free-dim row size per node (256)
    ctot = sum(nch)
    ctot2 = max(1, sum(m2ch))
    ctot3 = max(1, sum(m3ch))
    coloff = np.concatenate([[0], np.cumsum(nch)]).astype(int)
    coloff2 = np.concatenate([[0], np.cumsum(m2ch)]).astype(int)
    coloff3 = np.concatenate([[0], np.cumsum(m3ch)]).astype(int)
    NBF = N // P  # full blocks
    rem = N - NBF * P

    nc = bacc.Bacc("TRN2", target_bir_lowering=False, debug=False, num_swdge_queues=2)

    x_in = nc.dram_tensor("x", [BPC, N, H], F32, kind="ExternalInput").ap()
    m_in = nc.dram_tensor("mask", [BPC, N], I32, kind="ExternalInput").ap()
    w1_in = nc.dram_tensor("W1", [2 * H, H], F32, kind="ExternalInput").ap()
    b1_in = nc.dram_tensor("b1", [H], F32, kind="ExternalInput").ap()
    w2_in = nc.dram_tensor("W2", [H, H], F32, kind="ExternalInput").ap()
    b2_in = nc.dram_tensor("b2", [H], F32, kind="ExternalInput").ap()
    idx_in = nc.dram_tensor("idx", [P, ctot * 8], I16, kind="ExternalInput").ap()
    dl_in = nc.dram_tensor("dl", [P, ctot, 2], BF16, kind="ExternalInput").ap()
    dl2_in = nc.dram_tensor("dl2", [P, ctot2, 2], BF16, kind="ExternalInput").ap()
    dl3_in = nc.dram_tensor("dl3", [P, ctot3, 2], BF16, kind="ExternalInput").ap()
    iota_in = nc.dram_tensor("iota", [P, P], BF16, kind="ExternalInput").ap()
    idb_in = nc.dram_tensor("idb", [P, P], BF16, kind="ExternalInput").ap()
    idf_in = nc.dram_tensor("idf", [P, P], F32, kind="ExternalInput").ap()
    y_out = nc.dram_tensor("y", [BPC, N, H], F32, kind="ExternalOutput").ap()

    with tile.TileContext(nc) as tc:
        with (
            tc.tile_pool(name="const", bufs=1) as cpool,
            tc.tile_pool(name="work", bufs=3) as wpool,
            tc.tile_pool(name="msgs", bufs=2) as mpool,
            tc.tile_pool(name="ohp", bufs=2) as opool,
            tc.tile_pool(name="idxp", bufs=3) as ipool,
            tc.tile_pool(name="psA", bufs=2, space="PSUM") as psA,
            tc.tile_pool(name="psB", bufs=1, space="PSUM") as psB,
            tc.tile_pool(name="dram", bufs=1, space="DRAM") as dpool,
        ):
            # ---- constants / persistent tiles ----
            dl_t = cpool.tile([P, ctot, 2], BF16, tag="dl")
            nc.sync.dma_start(out=dl_t[:], in_=dl_in)
            dl2_t = cpool.tile([P, ctot2, 2], BF16, tag="dl2")
            nc.sync.dma_start(out=dl2_t[:], in_=dl2_in)
            dl3_t = cpool.tile([P, ctot3, 2], BF16, tag="dl3")
            nc.sync.dma_start(out=dl3_t[:], in_=dl3_in)
            iota_t = cpool.tile([P, P], BF16, tag="iota")
            nc.sync.dma_start(out=iota_t[:], in_=iota_in)
            idb_t = cpool.tile([P, P], BF16, tag="idb")
            nc.sync.dma_start(out=idb_t[:], in_=idb_in)
            idf_t = cpool.tile([P, P], F32, tag="idf")
            nc.sync.dma_start(out=idf_t[:], in_=idf_in)

            w1a_f = cpool.tile([P, H], F32, tag="w1af")
            nc.sync.dma_start(out=w1a_f[:], in_=w1_in[0:H, :])
            w1b_f = cpool.tile([P, H], F32, tag="w1bf")
            nc.sync.dma_start(out=w1b_f[:], in_=w1_in[H:2 * H, :])
            w2_f = cpool.tile([P, H], F32, tag="w2f")
            nc.sync.dma_start(out=w2_f[:], in_=w2_in)

            b1c = cpool.tile([P, 1], F32, tag="b1c")
            nc.sync.dma_start(out=b1c[:], in_=b1_in[:, None])
            b2c = cpool.tile([P, 1], F32, tag="b2c")
            nc.sync.dma_start(out=b2c[:], in_=b2_in[:, None])

            # mask -> per-block per-partition columns, f32, and (1-m)
            mi = cpool.tile([P, NB, BPC], I32, tag="mi")
            for j in range(BPC):
                nc.sync.dma_start(
                    out=mi[:, 0:NBF, j],
                    in_=m_in[j, 0:NBF * P].rearrange("(nb p) -> p nb", p=P),
                )
                if rem:
                    nc.sync.dma_start(
                        out=mi[:rem, NBF, j], in_=m_in[j, NBF * P:N][:, None]
                    )
            m_f = cpool.tile([P, NB, BPC], F32, tag="mf")
            nc.vector.tensor_copy(out=m_f[:], in_=mi[:])
            omm = cpool.tile([P, NB, BPC], F32, tag="omm")
            nc.vector.tensor_scalar(
                out=omm[:], in0=m_f[:], scalar1=-1.0, scalar2=1.0,
                op0=ALU.mult, op1=ALU.add,
            )

            # resident fp32 x, node-major per block: [p, nb, batch, h].
            # Loaded in row ranges; each range is cast to the bf16 gather
            # table as soon as its batches land, overlapping load and cast.
            x_res = cpool.tile([P, NB, BPC, H], F32, tag="xres")
            xbf = dpool.tile([NPAD, BPC, H], BF16, tag="xbf")
            RSPLIT = 4
            edges_r = [NBF * (i + 1) // RSPLIT for i in range(RSPLIT)]
            lo = 0
            for hi in edges_r:
                for j in range(BPC):
                    nc.sync.dma_start(
                        out=x_res[:, lo:hi, j, :],
                        in_=x_in[j, lo * P:hi * P, :].rearrange(
                            "(nb p) h -> p nb h", p=P
                        ),
                    )
                nc.gpsimd.dma_start(
                    out=xbf[lo * P:hi * P, :, :].rearrange(
                        "(nb p) b h -> p nb (b h)", p=P
                    ),
                    in_=x_res[:, lo:hi, :, :].rearrange("p nb b h -> p nb (b h)"),
                )
                lo = hi
            if rem:
                for j in range(BPC):
                    nc.sync.dma_start(
                        out=x_res[:rem, NBF, j, :],
                        in_=x_in[j, NBF * P:N, :],
                    )
                nc.gpsimd.dma_start(
                    out=xbf[NBF * P:N, :, :].rearrange("p b h -> p (b h)"),
                    in_=x_res[:rem, NBF, :, :].rearrange("p b h -> p (b h)"),
                )
            zrow = cpool.tile([16, FD], BF16, tag="zrow")
            nc.gpsimd.memset(zrow[:], 0.0)
            nc.sync.dma_start(
                out=xbf[N:NPAD, :, :].rearrange("r b h -> r (b h)"), in_=zrow[:]
            )

            # ---- main loop over dst blocks (gathers merged in pairs) ----
            xbf_flat = xbf[:].rearrange("r b h -> r (b h)")
            IDXG = 4  # blocks per idx-load DMA
            idx_tiles = {}
            pairs = [
                tuple(pb_ for pb_ in (b0, b0 + 1) if pb_ < NB)
                for b0 in range(0, NB, 2)
            ]
            for pair in pairs:
                b0 = pair[0]
                kp = sum(nch[b] for b in pair)
                poff = int(coloff[b0])

                if b0 % IDXG == 0:
                    ge = min(NB, b0 + IDXG)
                    goff = int(coloff[b0])
                    gk = int(coloff[ge]) - goff
                    gt = ipool.tile([P, gk * 8], I16, tag="idxs")
                    nc.sync.dma_start(
                        out=gt[:], in_=idx_in[:, goff * 8:(goff + gk) * 8]
                    )
                    idx_tiles = {"tile": gt, "goff": goff}
                loc = (poff - idx_tiles["goff"]) * 8
                ms = mpool.tile([P, kp, FD], BF16, tag="ms")
                nc.gpsimd.dma_gather(
                    out_ap=ms[:],
                    in_ap=xbf_flat,
                    idxs_ap=idx_tiles["tile"][:, loc:loc + kp * 8],
                    num_idxs=kp * P,
                    num_idxs_reg=kp * P,
                    elem_size=FD,
                    single_packet=False,
                    queue_num=(b0 // 2) % 2,
                )
                oh = opool.tile([P, kp, P], BF16, tag="oh")
                nc.vector.tensor_tensor(
                    out=oh[:].rearrange("p c (a t) -> p c a t", t=2),
                    in0=iota_t[:, None, :].to_broadcast([P, kp, P]).rearrange(
                        "p c (a t) -> p c a t", t=2
                    ),
                    in1=dl_t[:, poff:poff + kp, None, :].to_broadcast(
                        [P, kp, P // 2, 2]
                    ),
                    op=ALU.is_equal,
                )

                for b in pair:
                    lo = int(coloff[b]) - poff
                    for mch, dlx_t, cofx, tg in (
                        (m2ch[b], dl2_t, coloff2, "oh2"),
                        (m3ch[b], dl3_t, coloff3, "oh3"),
                    ):
                        if not mch:
                            continue
                        offx = int(cofx[b])
                        tx = opool.tile([P, mch, P], BF16, tag=tg)
                        nc.vector.tensor_tensor(
                            out=tx[:].rearrange("p c (a t) -> p c a t", t=2),
                            in0=iota_t[:, None, :].to_broadcast(
                                [P, mch, P]
                            ).rearrange("p c (a t) -> p c a t", t=2),
                            in1=dlx_t[:, offx:offx + mch, None, :].to_broadcast(
                                [P, mch, P // 2, 2]
                            ),
                            op=ALU.is_equal,
                        )
                        nc.vector.tensor_tensor(
                            out=oh[:, lo:lo + mch, :],
                            in0=oh[:, lo:lo + mch, :],
                            in1=tx[:],
                            op=ALU.add,
                        )

                for b in pair:
                    c0 = b * P
                    pb = min(P, N - c0)
                    k = nch[b]
                    lo = int(coloff[b]) - poff

                    nbp = psA.tile([P, BPC, H], F32, tag="nbp", space="PSUM")
                    for c in range(k):
                        nc.tensor.matmul(
                            out=nbp[:].rearrange("p b h -> p (b h)"),
                            lhsT=oh[:, lo + c, :],
                            rhs=ms[:, lo + c, :],
                            start=(c == 0),
                            stop=(c == k - 1),
                        )
                    nb_sb = wpool.tile([P, BPC, H], F32, tag="nb_sb")
                    nc.scalar.activation(out=nb_sb[:], in_=nbp[:], func=AF.Copy)

                    tpn = psB.tile([P, BPC, P], F32, tag="tpn", space="PSUM")
                    for j in range(BPC):
                        nc.tensor.transpose(
                            out=tpn[:, j, :], in_=nb_sb[:, j, :], identity=idf_t[:]
                        )
                    nT = wpool.tile([P, BPC, P], F32, tag="nT")
                    nc.scalar.activation(out=nT[:], in_=tpn[:], func=AF.Copy)

                    # per-block feature-major x from resident fp32 x
                    tpx = psB.tile([P, BPC, P], F32, tag="tpx", space="PSUM")
                    for j in range(BPC):
                        nc.tensor.transpose(
                            out=tpx[:, j, :pb], in_=x_res[:pb, b, j, :],
                            identity=idf_t[:pb, :pb],
                        )
                    xTb = wpool.tile([P, BPC, P], F32, tag="xTb")
                    nc.scalar.activation(
                        out=xTb[:, :, :pb], in_=tpx[:, :, :pb], func=AF.Copy
                    )

                    h_ps = psB.tile([P, BPC, H], F32, tag="h_ps", space="PSUM")
                    nc.tensor.matmul(
                        out=h_ps[:, :, :pb], lhsT=w1a_f[:], rhs=nT[:, :, :pb],
                        start=True, stop=False,
                    )
                    nc.tensor.matmul(
                        out=h_ps[:, :, :pb], lhsT=w1b_f[:], rhs=xTb[:, :, :pb],
                        start=False, stop=True,
                    )
                    h_sb = wpool.tile([P, BPC, H], F32, tag="h_sb")
                    nc.scalar.activation(
                        out=h_sb[:, :, :pb], in_=h_ps[:, :, :pb], func=AF.Relu,
                        bias=b1c[:],
                    )

                    imp_ps = psB.tile([P, BPC, H], F32, tag="imp_ps", space="PSUM")
                    nc.tensor.matmul(
                        out=imp_ps[:, :, :pb], lhsT=w2_f[:],
                        rhs=h_sb[:, :, :pb],
                        start=True, stop=True,
                    )
                    imp_sb = wpool.tile([P, BPC, H], F32, tag="imp_sb")
                    nc.vector.tensor_scalar(
                        out=imp_sb[:, :, :pb], in0=imp_ps[:, :, :pb],
                        scalar1=b2c[:], scalar2=None, op0=ALU.add,
                    )

                    outT = psA.tile([P, BPC, P], F32, tag="outT", space="PSUM")
                    for j in range(BPC):
                        nc.tensor.transpose(
                            out=outT[:pb, j, :], in_=imp_sb[:, j, :pb],
                            identity=idf_t[:],
                        )

                    xt_sb = wpool.tile([P, BPC, H], F32, tag="xt_sb")
                    for j in range(BPC):
                        nc.scalar.activation(
                            out=xt_sb[:pb, j, :], in_=x_res[:pb, b, j, :],
                            func=AF.Copy, scale=omm[:pb, b:b + 1, j],
                        )
                    out_sb = wpool.tile([P, BPC, H], F32, tag="out_sb")
                    for j in range(BPC):
                        nc.vector.scalar_tensor_tensor(
                            out=out_sb[:pb, j, :],
                            in0=outT[:pb, j, :],
                            scalar=m_f[:pb, b:b + 1, j],
                            in1=xt_sb[:pb, j, :],
                            op0=ALU.mult,
                            op1=ALU.add,
                        )
                    nc.sync.dma_start(
                        out=y_out[:, c0:c0 + pb, :].rearrange("b p h -> p b h"),
                        in_=out_sb[:pb],
                    )

    nc.compile()
    return nc


def kernel(node_embeddings, missing_mask, edge_index, W1, b1, W2, b2, trace=False):
    global last_results
    x = np.ascontiguousarray(np.asarray(node_embeddings, np.float32))
    mask = np.ascontiguousarray(np.asarray(missing_mask, np.int32))
    B, N, H = x.shape
    assert H == P and B % NCORES == 0
    BPC = B // NCORES

    ekey = (N, B, H, hash(np.asarray(edge_index).tobytes()),
            hash(mask.tobytes()))
    if ekey not in _cache:
        keeps = [
            (mask[c * BPC:(c + 1) * BPC] != 0).any(axis=0) for c in range(NCORES)
        ]
        nch, m2ch, m3ch, idx_list, dl1_list, dl2_list, dl3_list = _prep_edges(
            edge_index, N, keeps
        )
        nc = _build(nch, m2ch, m3ch, B, N, H)
        _cache[ekey] = (nc, idx_list, dl1_list, dl2_list, dl3_list)
    nc, idx_list, dl1_list, dl2_list, dl3_list = _cache[ekey]

    iota_arr = np.broadcast_to(
        np.arange(P, dtype=np.float32), (P, P)
    ).astype(NPBF16).copy()
    idb_arr = np.eye(P, dtype=np.float32).astype(NPBF16)
    idf_arr = np.eye(P, dtype=np.float32)

    common = {
        "W1": np.ascontiguousarray(np.asarray(W1, np.float32)),
        "b1": np.ascontiguousarray(np.asarray(b1, np.float32)),
        "W2": np.ascontiguousarray(np.asarray(W2, np.float32)),
        "b2": np.ascontiguousarray(np.asarray(b2, np.float32)),
        "iota": iota_arr,
        "idb": idb_arr,
        "idf": idf_arr,
    }
    in_maps = []
    for c in range(NCORES):
        m = dict(common)
        m["x"] = np.ascontiguousarray(x[c * BPC:(c + 1) * BPC])
        m["mask"] = np.ascontiguousarray(mask[c * BPC:(c + 1) * BPC])
        m["idx"] = idx_list[c]
        m["dl"] = dl1_list[c]
        m["dl2"] = dl2_list[c]
        m["dl3"] = dl3_list[c]
        in_maps.append(m)

    try:
        res = bass_utils.run_bass_kernel_spmd(
            nc, in_maps, core_ids=list(range(NCORES)), trace=trace
        )
    except ModuleNotFoundError:
        res = bass_utils.run_bass_kernel_spmd(
            nc, in_maps, core_ids=list(range(NCORES)), trace=False
        )
    last_results = res
    return np.concatenate([res.results[c]["y"] for c in range(NCORES)], axis=0)
